# revision 21
# baseline (speedup 1.0000x reference)
"""GATNE-T inference kernel for 8 Trainium2 NeuronCores.

Data-parallel over the batch (1024 samples/core), tables replicated.

The dominant cost is gathering 40 random embedding rows per sample.
Per-row indirect DMAs cost ~1us of SWDGE (Q7) time each => ~350us/core.
Instead, the batched MoE primitives are used with host-side index prep:

  1. dma_gather: requests sorted by (table-region, batch-half, edge-type)
     on the host; one instruction per 32768-row region gathers full 256B
     bf16 rows with int16 region-local indices.
  2. staging: per (region,half,type), the type's 32-col slice is copied to
     a contiguous staging tile (scalar engine).
  3. dma_scatter_add: one instruction per (region,half) writes each slice
     to its own unique row (b%512)*40 + t*10 + s of a zeroed DRAM buffer
     (collisions in scatter-add lose updates on TRN2 -- verified -- so the
     destination rows are unique per request; capacity padding goes to
     trash rows).
  4. one strided reload per (tile, type) + a vector-engine sum over the
     S=10 neighbor slots rebuilds node_agg per 128-sample tile.

The 1/S neighbor-mean is folded into the s1/tw weights on the host, the
per-type attention paths are evaluated with block-diagonal 128x128
matmuls, and the sample's type is selected with host-built one-hot masks.
"""
import sys

sys.path.insert(0, "/opt/trn_rl_repo")

import numpy as np

import concourse.bass as bass
import concourse.tile as tile
from concourse import bacc, mybir
from concourse.bass_utils import run_bass_kernel_spmd
from concourse.masks import make_identity

V = 500000
T = 4
D = 32
E = 128
A = 32
B = 8192
S = 10
NCORES = 8
BL = B // NCORES          # 1024 samples per core
P = 128                   # partitions / samples per tile
NTILES = BL // P          # 8 tiles per core
F32 = mybir.dt.float32
BF16 = mybir.dt.bfloat16
I32 = mybir.dt.int32
I16 = mybir.dt.int16

REGROWS = 32768                       # int16-addressable rows per region
NREG = (V + REGROWS - 1) // REGROWS   # 16 (last region has 8480 rows)
HB = BL // 2                          # batch half (512 samples)
# static per-(region, half, type) subgroup capacity, multiple of 128
CAPS = [512] * (NREG - 1) + [128]
RBASE = np.cumsum([0] + [8 * c for c in CAPS]).tolist()
NSLOT = RBASE[-1]                     # 62464
NDST = HB * T * S                     # 20480 rows per half-buffer
NTRASH = 128

_cache = {}


def _build():
    nc = bacc.Bacc("TRN2", target_bir_lowering=False, debug=False,
                   num_devices=NCORES)
    nte = nc.dram_tensor("nte", [V, T * D], BF16, kind="ExternalInput").ap()
    base = nc.dram_tensor("base", [V, E], F32, kind="ExternalInput").ap()
    tw = nc.dram_tensor("tw", [T * D, E], F32, kind="ExternalInput").ap()
    s1bd_in = nc.dram_tensor("s1bd", [P, T * P], F32,
                             kind="ExternalInput").ap()
    s2bd_in = nc.dram_tensor("s2bd", [P, T * T], F32,
                             kind="ExternalInput").ap()
    tgts = nc.dram_tensor("tgts", [BL, 1], I32, kind="ExternalInput").ap()
    msk = nc.dram_tensor("msk", [BL, T], F32, kind="ExternalInput").ap()
    gidx = nc.dram_tensor("gidx", [P, NSLOT // 16], I16,
                          kind="ExternalInput").ap()
    sidx = nc.dram_tensor("sidx", [P, NSLOT // 16], I16,
                          kind="ExternalInput").ap()
    perm = [nc.dram_tensor(f"perm{h}", [NDST + NTRASH, T * D], BF16)
            for h in range(2)]
    out = nc.dram_tensor("out", [BL, E], F32, kind="ExternalOutput").ap()

    with tile.TileContext(nc) as tc:
        _emit(tc, nc, nte, base, tw, s1bd_in, s2bd_in, tgts, msk,
              gidx, sidx, perm, out)
    nc.compile()
    return nc


def _emit(tc, nc, nte, base, tw, s1bd_in, s2bd_in, tgts, msk,
          gidx, sidx, perm, out):
    import contextlib

    ctx = contextlib.ExitStack()
    with ctx:
        const = ctx.enter_context(tc.tile_pool(name="const", bufs=1))
        gpool = ctx.enter_context(tc.tile_pool(name="g", bufs=2))
        spool = ctx.enter_context(tc.tile_pool(name="s", bufs=2))
        ppool = ctx.enter_context(tc.tile_pool(name="p", bufs=2, space="PSUM"))

        # ---- zero the permutation buffers (scatter-add needs 0 base) ---
        # (NDST+NTRASH) rows = 20608 = 128 * 161 = 128 * 7 * 23
        zero_t = const.tile([P, 23 * T * D], BF16)
        nc.vector.memset(zero_t[:], 0.0)
        for h in range(2):
            pv = perm[h].ap().rearrange("(c a p) e -> c p a e", p=P, a=23)
            for c in range(7):
                nc.sync.dma_start(
                    out=pv[c],
                    in_=zero_t[:].rearrange("p (a e) -> p a e", a=23))

        # ---- constants / weights ---------------------------------------
        ident = const.tile([P, P], F32)
        make_identity(nc, ident[:])
        s1bd = const.tile([P, T * P], F32)
        nc.sync.dma_start(out=s1bd[:], in_=s1bd_in[:])
        s2bd = const.tile([P, T * T], F32)
        nc.sync.dma_start(out=s2bd[:], in_=s2bd_in[:])
        twstack = const.tile([P, E], F32)
        nc.sync.dma_start(out=twstack[:], in_=tw[:])

        tgt_all = const.tile([P, NTILES], I32)
        nc.sync.dma_start(
            out=tgt_all[:],
            in_=tgts.rearrange("(i p) o -> p (i o)", p=P))
        msk_all = const.tile([P, NTILES, T], F32)
        nc.sync.dma_start(
            out=msk_all[:],
            in_=msk.rearrange("(i p) j -> p i j", p=P))
        gidx_t = const.tile([P, NSLOT // 16], I16)
        nc.sync.dma_start(out=gidx_t[:], in_=gidx[:])
        sidx_t = const.tile([P, NSLOT // 16], I16)
        nc.sync.dma_start(out=sidx_t[:], in_=sidx[:])

        # ---- base embeddings: proven per-partition indirect gathers ----
        base_all = const.tile([P, NTILES, E], F32)
        for i in range(NTILES):
            nc.gpsimd.indirect_dma_start(
                out=base_all[:, i, :], out_offset=None, in_=base[:],
                in_offset=bass.IndirectOffsetOnAxis(
                    ap=tgt_all[:, i:i + 1], axis=0))

        # ---- neighbor rows: gather -> stage slices -> scatter(write) ---
        for r in range(NREG):
            cap = CAPS[r]
            nidx = 8 * cap              # 2 halves x 4 types
            nrows = min(REGROWS, V - r * REGROWS)
            c0 = RBASE[r] // 16
            gbuf = gpool.tile([P, nidx // P, T * D], BF16, tag="gbuf")
            nc.gpsimd.dma_gather(
                out_ap=gbuf[:], in_ap=nte[r * REGROWS: r * REGROWS + nrows, :],
                idxs_ap=gidx_t[:, c0:c0 + nidx // 16],
                num_idxs=nidx, num_idxs_reg=nidx, elem_size=T * D,
                single_packet=False)
            w = cap // P
            for h in range(2):
                stage = gpool.tile([P, 4 * w, D], BF16, tag="stage")
                for t in range(T):
                    g0 = (h * T + t) * w
                    nc.scalar.copy(
                        stage[:, t * w:(t + 1) * w, :],
                        gbuf[:, g0:g0 + w, t * D:(t + 1) * D])
                sc0 = c0 + h * T * cap // 16
                nc.gpsimd.dma_scatter_add(
                    out_ap=perm[h].ap()[:, 0:D],
                    in_ap=stage[:],
                    idxs_ap=sidx_t[:, sc0:sc0 + 4 * cap // 16],
                    num_idxs=4 * cap, num_idxs_reg=4 * cap,
                    elem_size=D, elem_step=T * D, single_packet=False)

        # ---- per-tile: reload neighbor slices, reduce, attention -------
        for i in range(NTILES):
            r0 = i * P
            h = i // (NTILES // 2)
            # rows (b%512)*40 + t*10 + s, col t*32+d
            pb = perm[h].ap()[0:NDST, :].rearrange(
                "(p j) e -> p j e", j=T * S)  # p = b%512 here
            g = spool.tile([P, T, S, D], BF16, tag="g")
            for t in range(T):
                nc.sync.dma_start(
                    out=g[:, t, :, :],
                    in_=pb[(i % (NTILES // 2)) * P:
                           (i % (NTILES // 2)) * P + P,
                           t * S:(t + 1) * S, 0:D])
            agg = spool.tile([P, T * D], F32, tag="agg")
            nc.vector.reduce_sum(
                agg[:], g[:].rearrange("p t s d -> p t d s"),
                axis=mybir.AxisListType.X)

            aggT_p = ppool.tile([P, P], F32, tag="mm")
            nc.tensor.transpose(out=aggT_p[:], in_=agg[:], identity=ident[:])
            aggT = spool.tile([P, P], F32, tag="aggT_s")
            nc.scalar.copy(aggT[:], aggT_p[:])

            sc_p = ppool.tile([P, T * T], F32, tag="sc")
            for w in range(T):
                u_p = ppool.tile([P, P], F32, tag="u")
                nc.tensor.matmul(u_p[:], lhsT=s1bd[:, w * P:(w + 1) * P],
                                 rhs=aggT[:], start=True, stop=True)
                u_s = spool.tile([P, P], F32, tag="u_s")
                nc.scalar.activation(u_s[:], u_p[:],
                                     mybir.ActivationFunctionType.Tanh)
                nc.tensor.matmul(sc_p[:, w * T:(w + 1) * T], lhsT=u_s[:],
                                 rhs=s2bd[:, w * T:(w + 1) * T],
                                 start=True, stop=True)

            scm = spool.tile([P, T * T], F32, tag="scm")
            nc.vector.tensor_tensor(
                out=scm[:].rearrange("p (w t) -> p w t", w=T),
                in0=sc_p[:].rearrange("p (w t) -> p w t", w=T),
                in1=msk_all[:, i, :, None].to_broadcast([P, T, T]),
                op=mybir.AluOpType.mult)
            scsel = spool.tile([P, T], F32, tag="scsel")
            nc.vector.reduce_sum(
                scsel[:], scm[:].rearrange("p (w t) -> p t w", w=T),
                axis=mybir.AxisListType.X)

            ex = spool.tile([P, T], F32, tag="ex")
            sm = spool.tile([P, 1], F32, tag="sm")
            nc.scalar.activation(ex[:], scsel[:],
                                 mybir.ActivationFunctionType.Exp,
                                 accum_out=sm[:])
            inv = spool.tile([P, 1], F32, tag="inv")
            nc.vector.reciprocal(inv[:], sm[:])

            minv = spool.tile([P, T], F32, tag="minv")
            nc.vector.tensor_tensor(
                out=minv[:], in0=msk_all[:, i, :],
                in1=inv[:, 0:1].to_broadcast([P, T]),
                op=mybir.AluOpType.mult)
            prod = spool.tile([P, T * D], F32, tag="prod")
            nc.vector.tensor_tensor(
                out=prod[:].rearrange("p (t d) -> p t d", t=T),
                in0=agg[:].rearrange("p (t d) -> p t d", t=T),
                in1=ex[:, :, None].to_broadcast([P, T, D]),
                op=mybir.AluOpType.mult)
            natt = spool.tile([P, D], F32, tag="natt")
            nc.vector.reduce_sum(
                natt[:], prod[:].rearrange("p (t d) -> p d t", t=T),
                axis=mybir.AxisListType.X)
            natt4 = spool.tile([P, T * D], F32, tag="natt4")
            nc.vector.tensor_tensor(
                out=natt4[:].rearrange("p (w d) -> p w d", w=T),
                in0=natt[:, None, :].to_broadcast([P, T, D]),
                in1=minv[:, :, None].to_broadcast([P, T, D]),
                op=mybir.AluOpType.mult)

            natt4T_p = ppool.tile([P, P], F32, tag="mm")
            nc.tensor.transpose(out=natt4T_p[:], in_=natt4[:],
                                identity=ident[:])
            natt4T = spool.tile([P, P], F32, tag="natt4T_s")
            nc.scalar.copy(natt4T[:], natt4T_p[:])

            proj_p = ppool.tile([P, E], F32, tag="mm")
            nc.tensor.matmul(proj_p[:], lhsT=natt4T[:], rhs=twstack[:],
                             start=True, stop=True)
            sumv = spool.tile([P, E], F32, tag="sumv")
            nc.vector.tensor_add(sumv[:], base_all[:, i, :], proj_p[:])
            sq = spool.tile([P, E], F32, tag="sq")
            ssum = spool.tile([P, 1], F32, tag="ssum")
            nc.scalar.activation(sq[:], sumv[:],
                                 mybir.ActivationFunctionType.Square,
                                 accum_out=ssum[:])
            sr = spool.tile([P, 1], F32, tag="sr")
            nc.scalar.activation(sr[:], ssum[:],
                                 mybir.ActivationFunctionType.Sqrt)
            rs = spool.tile([P, 1], F32, tag="rs")
            nc.vector.reciprocal(rs[:], sr[:])
            res = spool.tile([P, E], F32, tag="res")
            nc.scalar.mul(res[:], sumv[:], rs[:, 0:1])
            nc.sync.dma_start(out=out[r0:r0 + P, :], in_=res[:])


def get_nc():
    if "nc" not in _cache:
        _cache["nc"] = _build()
    return _cache["nc"]


def _wrap16(a):
    """int16 list -> [128, len/16] wrapped (idx i at [i%16, i//16]),
    replicated across the 8 gpsimd core partition groups."""
    return np.ascontiguousarray(np.tile(a.reshape(-1, 16).T, (8, 1)))


def prep_in_maps(targets, types, neighbors, base_node_embeddings,
                 node_type_embeddings, trans_weights, trans_weights_s1,
                 trans_weights_s2):
    bf16 = mybir.dt.np(BF16)
    targets = np.asarray(targets, dtype=np.int32)
    types = np.asarray(types, dtype=np.int32)
    neighbors = np.asarray(neighbors, dtype=np.int64)
    nte = np.ascontiguousarray(np.asarray(
        node_type_embeddings, dtype=np.float32)).reshape(V, T * D)
    nte_bf = np.ascontiguousarray(nte.astype(bf16))
    basearr = np.ascontiguousarray(
        np.asarray(base_node_embeddings, dtype=np.float32)).reshape(V, E)
    tw = np.ascontiguousarray(
        np.asarray(trans_weights, dtype=np.float32) / S).reshape(T * D, E)
    s1 = np.asarray(trans_weights_s1, dtype=np.float32) / S
    s2 = np.asarray(trans_weights_s2, dtype=np.float32).reshape(T, A)

    s1bd = np.zeros((P, T * P), dtype=np.float32)
    s2bd = np.zeros((P, T * T), dtype=np.float32)
    for w in range(T):
        for t in range(T):
            s1bd[t * D:(t + 1) * D, w * P + t * A:w * P + (t + 1) * A] = s1[w]
            s2bd[t * A:(t + 1) * A, w * T + t] = s2[w]

    masks = np.zeros((B, T), dtype=np.float32)
    masks[np.arange(B), types] = 1.0

    t_of = np.repeat(np.arange(T, dtype=np.int64), S)       # [40]
    in_maps = []
    for c in range(NCORES):
        sl = slice(c * BL, (c + 1) * BL)
        v = neighbors[sl].reshape(BL, T * S)                # [BL, 40]
        b_idx = np.broadcast_to(
            np.arange(BL, dtype=np.int64)[:, None], v.shape)
        j_idx = np.broadcast_to(
            np.arange(T * S, dtype=np.int64)[None, :], v.shape)
        vf = v.ravel()
        bf = b_idx.ravel()
        jf = j_idx.ravel()
        tf = t_of[jf]
        reg = vf >> 15
        half = bf // HB
        key = (reg * 2 + half) * T + tf
        order = np.argsort(key, kind="stable")
        counts = np.bincount(key, minlength=NREG * 8)

        gl = np.zeros(NSLOT, dtype=np.int16)
        sl_list = np.zeros(NSLOT, dtype=np.int16)
        pos = 0
        for k in range(NREG * 8):
            r = k // 8
            cap = CAPS[r]
            cnt = int(counts[k])
            assert cnt <= cap, (k, cnt, cap)
            grp = order[pos:pos + cnt]
            pos += cnt
            b0 = RBASE[r] + (k % 8) * cap
            gl[b0:b0 + cnt] = (vf[grp] - r * REGROWS).astype(np.int16)
            gl[b0 + cnt:b0 + cap] = gl[b0] if cnt else 0
            sl_list[b0:b0 + cnt] = (
                (bf[grp] % HB) * (T * S) + jf[grp]).astype(np.int16)
            npad = cap - cnt
            sl_list[b0 + cnt:b0 + cap] = (
                NDST + (np.arange(npad) % NTRASH)).astype(np.int16)

        in_maps.append({
            "nte": nte_bf,
            "base": basearr,
            "tw": tw,
            "s1bd": s1bd,
            "s2bd": s2bd,
            "tgts": np.ascontiguousarray(targets[sl][:, None]),
            "msk": np.ascontiguousarray(masks[sl]),
            "gidx": _wrap16(gl),
            "sidx": _wrap16(sl_list),
        })
    return in_maps


def kernel(targets, types, neighbors, base_node_embeddings,
           node_type_embeddings, trans_weights, trans_weights_s1,
           trans_weights_s2):
    nc = get_nc()
    in_maps = prep_in_maps(targets, types, neighbors, base_node_embeddings,
                           node_type_embeddings, trans_weights,
                           trans_weights_s1, trans_weights_s2)
    res = run_bass_kernel_spmd(nc, in_maps, core_ids=list(range(NCORES)))
    return np.concatenate([res.results[c]["out"] for c in range(NCORES)],
                          axis=0)
